# revision 35
# baseline (speedup 1.0000x reference)
"""MoE routing kernel for Trainium2 (8 NeuronCores, expert-parallel).

Problem: nn_HDynMoF — hierarchical top-p MoE with 16 SwiGLU experts
(D=512 -> H=2048 -> D=512), 4 groups x 4 experts, top-2 groups (top-p 0.9),
top-2 experts per group (top-p 0.9), N = 2*1024 tokens.

Strategy:
  - Expert-parallel: core c owns experts {2c, 2c+1}; both live in group c//2.
  - Host permutes the router/gate columns per core so the local group sits in
    group-slot 0 and the local experts in expert-slots 0/1 -> one SPMD program.
  - Each core computes routing for all tokens (cheap) and the dense SwiGLU for
    its 2 experts over all tokens, weighted by the routing weight, then an
    8-core ReduceScatter(add) combines and shards the output over tokens.
"""

import os
import numpy as np

# Problem dims (hardcoded per contract - kernel.py is self-contained).
B, T, D, H = 2, 1024, 512, 2048
N = B * T               # 2048 tokens
G, EPG, E = 4, 4, 16
GTP, TP = 0.9, 0.9
SCALE = 0.5             # 1/sqrt(G)
NCORES = 8
EPC = E // NCORES       # experts per core
P = 128
KD = D // P             # 4 k-tiles over D
NT = N // P             # 16 token tiles
CHW = 512               # token chunk (matmul moving free dim, fp32 max)
CH = N // CHW           # 4 chunks
NH = 2                  # halves of H
HTH = H // NH // P      # 8 h-tiles per half
HW = H // NH            # 1024 cols per half
SHARD = N // NCORES     # 256

_PROG = None
LAST_EXEC_NS = None
LAST_TRACE = None


def _emit(tc, xt, xrow16, tokid, wcat, bcat, w1s, w3s, w2s, out_sh):
    import concourse.bass as bass
    import concourse.mybir as mybir
    from concourse.masks import make_identity

    nc = tc.nc
    f32 = mybir.dt.float32
    bf16 = mybir.dt.bfloat16  # expert path: PE 1 cyc/row (fp32 is 4)
    Alu = mybir.AluOpType
    Act = mybir.ActivationFunctionType
    AX = mybir.AxisListType

    with tc.tile_pool(name="cp", bufs=1) as cp:
        # ---- resident inputs -------------------------------------------------
        xt_t = []
        for k in range(KD):
            t = cp.tile([P, N], f32, name=f"xtt{k}", tag=f"xtt{k}")
            nc.sync.dma_start(t[:], xt[k * P:(k + 1) * P, :])
            xt_t.append(t)
        wc_t = []
        for k in range(KD):
            t = cp.tile([P, G + E], f32, name=f"wct{k}", tag=f"wct{k}")
            nc.sync.dma_start(t[:], wcat[k * P:(k + 1) * P, :])
            wc_t.append(t)
        bc_t = cp.tile([1, G + E], f32, name="bct", tag="bct")
        nc.sync.dma_start(bc_t[:], bcat[:, :])
        ones_t = cp.tile([1, P], f32, name="onest", tag="onest")
        nc.vector.memset(ones_t[:], 1.0)

        # ---- routing logits, packed along free dim ---------------------------
        # L4:  (128, NT*4)  group logits, col = tile*4 + g
        # L16: (128, NT*16) expert logits, col = tile*16 + g*4 + e
        L4 = cp.tile([P, NT * G], f32, name="L4", tag="L4")
        L16 = cp.tile([P, NT * E], f32, name="L16", tag="L16")
        C20 = G + E
        with tc.tile_pool(name="rps", bufs=1, space="PSUM") as rp:
            psb = [rp.tile([P, 8 * C20], f32, tag=f"rpsb{b}", space="PSUM",
                           name=f"rpsb{b}") for b in range(2)]
            for tt in range(NT):
                ps = psb[tt // 8][:, (tt % 8) * C20:(tt % 8 + 1) * C20]
                for k in range(KD):
                    nc.tensor.matmul(
                        out=ps,
                        lhsT=xt_t[k][:, tt * P:(tt + 1) * P],
                        rhs=wc_t[k][:],
                        start=(k == 0), stop=False)
                # bias add via rank-1 matmul: ones^T @ bcat
                nc.tensor.matmul(out=ps, lhsT=ones_t[:], rhs=bc_t[:],
                                 start=False, stop=True)
            for b in range(2):
                v = psb[b][:].rearrange("p (t c) -> p t c", c=C20)
                nc.vector.tensor_copy(
                    out=L4[:, b * 8 * G:(b + 1) * 8 * G]
                    .rearrange("p (t g) -> p t g", g=G),
                    in_=v[:, :, 0:G])
                nc.vector.tensor_copy(
                    out=L16[:, b * 8 * E:(b + 1) * 8 * E]
                    .rearrange("p (t e) -> p t e", e=E),
                    in_=v[:, :, G:C20])

        # ---- hierarchical top-p weights --------------------------------------
        def topp_weights(Lt, nseg, thresh, nm):
            """Per segment of 4 along free dim: softmax, keep top-1 always and
            top-2 iff p1+p2 <= thresh, renormalize kept. Returns (128, nseg*4).
            """
            L3 = Lt[:].rearrange("p (s e) -> p s e", e=4)

            def stat(sname):
                return cp.tile([P, nseg, 1], f32, name=f"{nm}_{sname}",
                               tag=f"{nm}_{sname}")

            def bc(t):
                return t[:].to_broadcast([P, nseg, 4])

            mx = stat("mx")
            nc.vector.tensor_reduce(out=mx[:], in_=L3, axis=AX.X, op=Alu.max)
            Ew = cp.tile([P, nseg * 4], f32, name=f"{nm}_E", tag=f"{nm}_E")
            E3 = Ew[:].rearrange("p (s e) -> p s e", e=4)
            # E = exp(L - mx)
            nc.vector.scalar_tensor_tensor(out=E3, in0=bc(mx), scalar=-1.0,
                                           in1=L3, op0=Alu.mult, op1=Alu.add)
            nc.scalar.activation(out=Ew[:], in_=Ew[:], func=Act.Exp)
            sm = stat("sm")
            nc.vector.tensor_reduce(out=sm[:], in_=E3, axis=AX.X, op=Alu.add)
            inv = stat("inv")
            nc.vector.reciprocal(out=inv[:], in_=sm[:])
            Pt = cp.tile([P, nseg * 4], f32, name=f"{nm}_P", tag=f"{nm}_P")
            P3 = Pt[:].rearrange("p (s e) -> p s e", e=4)
            nc.vector.tensor_tensor(out=P3, in0=E3, in1=bc(inv), op=Alu.mult)
            # top-1 / top-2 masks
            m1 = stat("m1")
            nc.vector.tensor_reduce(out=m1[:], in_=P3, axis=AX.X, op=Alu.max)
            mk1 = cp.tile([P, nseg * 4], f32, name=f"{nm}_mk1", tag=f"{nm}_mk1")
            mk13 = mk1[:].rearrange("p (s e) -> p s e", e=4)
            nc.vector.tensor_tensor(out=mk13, in0=P3, in1=bc(m1), op=Alu.is_equal)
            Pm = cp.tile([P, nseg * 4], f32, name=f"{nm}_Pm", tag=f"{nm}_Pm")
            Pm3 = Pm[:].rearrange("p (s e) -> p s e", e=4)
            nc.vector.scalar_tensor_tensor(out=Pm3, in0=mk13, scalar=-1e9,
                                           in1=P3, op0=Alu.mult, op1=Alu.add)
            m2 = stat("m2")
            nc.vector.tensor_reduce(out=m2[:], in_=Pm3, axis=AX.X, op=Alu.max)
            mk2 = cp.tile([P, nseg * 4], f32, name=f"{nm}_mk2", tag=f"{nm}_mk2")
            mk23 = mk2[:].rearrange("p (s e) -> p s e", e=4)
            nc.vector.tensor_tensor(out=mk23, in0=Pm3, in1=bc(m2), op=Alu.is_equal)
            # act2 = (m1 + m2 <= thresh)
            a2 = stat("a2")
            nc.vector.tensor_add(out=a2[:], in0=m1[:], in1=m2[:])
            nc.vector.tensor_scalar(out=a2[:], in0=a2[:], scalar1=thresh,
                                    scalar2=None, op0=Alu.is_le)
            # denom = m1 + act2*m2 + 1e-9 ; dinv = 1/denom
            den = stat("den")
            nc.vector.tensor_tensor(out=den[:], in0=a2[:], in1=m2[:], op=Alu.mult)
            nc.vector.tensor_add(out=den[:], in0=den[:], in1=m1[:])
            nc.vector.tensor_scalar(out=den[:], in0=den[:], scalar1=1e-9,
                                    scalar2=None, op0=Alu.add)
            dinv = stat("dinv")
            nc.vector.reciprocal(out=dinv[:], in_=den[:])
            # w = (mk1 + act2*mk2) * P * dinv
            sel = cp.tile([P, nseg * 4], f32, name=f"{nm}_sel", tag=f"{nm}_sel")
            sel3 = sel[:].rearrange("p (s e) -> p s e", e=4)
            nc.vector.tensor_tensor(out=sel3, in0=mk23, in1=bc(a2), op=Alu.mult)
            nc.vector.tensor_add(out=sel3, in0=sel3, in1=mk13)
            Wt = cp.tile([P, nseg * 4], f32, name=f"{nm}_W", tag=f"{nm}_W")
            W3 = Wt[:].rearrange("p (s e) -> p s e", e=4)
            nc.vector.tensor_tensor(out=W3, in0=sel3, in1=P3, op=Alu.mult)
            nc.vector.tensor_tensor(out=W3, in0=W3, in1=bc(dinv), op=Alu.mult)
            return Wt

        WG = topp_weights(L4, NT, GTP, "g")        # (128, NT*4)   [tile, g]
        WE = topp_weights(L16, NT * G, TP, "e")    # (128, NT*16)  [tile, g, e]
        # tw[tile, g, e] = WG[tile, g] * WE[tile, g, e] * SCALE
        TW = cp.tile([P, NT * E], f32, name="TW", tag="TW")
        TW3 = TW[:].rearrange("p (s e) -> p s e", e=4)
        WGb = WG[:].rearrange("p (s u) -> p s u", u=1).to_broadcast([P, NT * G, 4])
        WE3 = WE[:].rearrange("p (s e) -> p s e", e=4)
        nc.vector.tensor_tensor(out=TW3, in0=WGb, in1=WE3, op=Alu.mult)
        nc.vector.tensor_scalar(out=TW[:], in0=TW[:], scalar1=SCALE,
                                scalar2=None, op0=Alu.mult)

        # ---- sparse dispatch: per local expert, compact active tokens --------
        i32 = mybir.dt.int32
        Cp = 768                 # capacity per expert (counts ~510 +- 15)
        NCT = Cp // P            # 6 gathered-row tiles
        CB = 384                 # gathered-token matmul chunk (free dim)
        NCB = Cp // CB           # 2
        TS = CB // P             # 3 token sub-tiles per chunk
        BIG = 65536.0

        ident = cp.tile([P, P], bf16, name="ident", tag="ident")
        make_identity(nc, ident[:])
        tok_t = cp.tile([P, NT], i32, name="tokt", tag="tokt")
        nc.sync.dma_start(tok_t[:], tokid[:, :])
        zNT = cp.tile([P, NT], f32, name="zNT", tag="zNT")
        nc.vector.memset(zNT[:], 0.0)
        z1p = cp.tile([1, P], f32, name="z1p", tag="z1p")
        nc.vector.memset(z1p[:], 0.0)
        zc_i = cp.tile([P, NCT], i32, name="zci", tag="zci")
        nc.vector.memset(zc_i[:], 0)

        TWv = TW[:].rearrange("p (t e) -> p t e", e=E)
        fwsl = []    # per expert: (P, NCT) int32 token id per compacted slot
        xgrows = []  # per expert: NCT gathered-x row tiles (P, D) bf16
        with tc.tile_pool(name="dp", bufs=1, space="DRAM") as dp:
            with tc.tile_pool(name="mp", bufs=1) as mp:
                for j in range(EPC):
                    # mask of tokens routed to this expert, (P, NT) over tiles
                    mask = mp.tile([P, NT], f32, name=f"mask{j}", tag=f"mk{j}")
                    nc.vector.tensor_scalar(out=mask[:], in0=TWv[:, :, j],
                                            scalar1=0.0, scalar2=None,
                                            op0=Alu.is_gt)
                    # two-level exclusive prefix sum -> rank in compacted list
                    incl = mp.tile([P, NT], f32, name=f"incl{j}", tag=f"ic{j}")
                    nc.vector.tensor_tensor_scan(
                        out=incl[:], data0=mask[:], data1=zNT[:], initial=0.0,
                        op0=Alu.add, op1=Alu.add)
                    excl = mp.tile([P, NT], f32, name=f"excl{j}", tag=f"ex{j}")
                    nc.vector.tensor_tensor(out=excl[:], in0=incl[:],
                                            in1=mask[:], op=Alu.subtract)
                    rsT = mp.tile([1, P], f32, name=f"rsT{j}", tag=f"rt{j}")
                    nc.sync.dma_start(rsT[:], incl[:, NT - 1:NT])
                    rsI = mp.tile([1, P], f32, name=f"rsI{j}", tag=f"ri{j}")
                    nc.vector.tensor_tensor_scan(
                        out=rsI[:], data0=rsT[:], data1=z1p[:], initial=0.0,
                        op0=Alu.add, op1=Alu.add)
                    rsE = mp.tile([1, P], f32, name=f"rsE{j}", tag=f"re{j}")
                    nc.vector.tensor_tensor(out=rsE[:], in0=rsI[:],
                                            in1=rsT[:], op=Alu.subtract)
                    offs = mp.tile([P, 1], f32, name=f"offs{j}", tag=f"of{j}")
                    nc.sync.dma_start(offs[:], rsE[:])
                    rank = mp.tile([P, NT], f32, name=f"rank{j}", tag=f"rk{j}")
                    nc.vector.tensor_scalar(out=rank[:], in0=excl[:],
                                            scalar1=offs[:], scalar2=None,
                                            op0=Alu.add)
                    # scatter token ids to fwd list (inactive pushed OOB)
                    rbig = mp.tile([P, NT], f32, name=f"rbig{j}", tag=f"rb{j}")
                    nc.vector.tensor_scalar(out=rbig[:], in0=rank[:],
                                            scalar1=BIG, scalar2=None,
                                            op0=Alu.add)
                    rsc = mp.tile([P, NT], f32, name=f"rsc{j}", tag=f"rs{j}")
                    nc.vector.scalar_tensor_tensor(
                        out=rsc[:], in0=mask[:], scalar=-BIG, in1=rbig[:],
                        op0=Alu.mult, op1=Alu.add)
                    rsci = mp.tile([P, NT], i32, name=f"rsci{j}", tag=f"rc{j}")
                    nc.vector.tensor_copy(out=rsci[:], in_=rsc[:])
                    fd = dp.tile([Cp, 1], i32, name=f"fwd{j}", tag=f"fw{j}",
                                 space="DRAM")
                    nc.sync.dma_start(
                        fd[:, :].rearrange("(ct p) u -> p (ct u)", p=P),
                        zc_i[:])
                    for t in range(NT):
                        nc.gpsimd.indirect_dma_start(
                            out=fd[:, :],
                            out_offset=bass.IndirectOffsetOnAxis(
                                ap=rsci[:, t:t + 1], axis=0),
                            in_=tok_t[:, t:t + 1], in_offset=None,
                            bounds_check=Cp - 1, oob_is_err=False)
                    fws = cp.tile([P, NCT], i32, name=f"fws{j}", tag=f"fs{j}")
                    nc.sync.dma_start(
                        fws[:],
                        fd[:, :].rearrange("(ct p) u -> p (ct u)", p=P))
                    fwsl.append(fws)
                    # gather x rows for this expert (transpose happens later,
                    # per-expert, so expert 1's doesn't block expert 0's FFN
                    # in PE program order)
                    xgj = []
                    for ct in range(NCT):
                        xg = cp.tile([P, D], bf16, name=f"xg{j}_{ct}",
                                     tag=f"xr{j}_{ct}")
                        nc.gpsimd.indirect_dma_start(
                            out=xg[:], out_offset=None,
                            in_=xrow16[:, :],
                            in_offset=bass.IndirectOffsetOnAxis(
                                ap=fws[:, ct:ct + 1], axis=0))
                        xgj.append(xg)
                    xgrows.append(xgj)

            # ---- expert FFN on compacted tokens + combine --------------------
            acc = [cp.tile([P, D], bf16, name=f"acc{i}", tag=f"acc{i}")
                   for i in range(NT)]
            og = [[cp.tile([P, D], bf16, name=f"og{j}_{i}", tag=f"og{j}_{i}")
                   for i in range(NCT)] for j in range(EPC)]
            zrow = cp.tile([P, D], bf16, name="zrow", tag="zrow")
            nc.vector.memset(zrow[:], 0.0)
            # dense per-expert output stagings, pre-zeroed (inactive tokens
            # must read as 0.0, not stale DRAM)
            sg = []
            for j in range(EPC):
                s = dp.tile([N, D], bf16, name=f"sg{j}", tag=f"sg{j}",
                            space="DRAM")
                for t in range(NT):
                    nc.sync.dma_start(s[t * P:(t + 1) * P, :], zrow[:])
                sg.append(s)
            with tc.tile_pool(name="wp", bufs=2) as wp, \
                 tc.tile_pool(name="yp", bufs=10) as yp, \
                 tc.tile_pool(name="gp", bufs=8) as gp, \
                 tc.tile_pool(name="pa", bufs=2, space="PSUM") as pa, \
                 tc.tile_pool(name="pb", bufs=4, space="PSUM") as pb:
                for j in range(EPC):
                    # transpose this expert's gathered x rows to (D, Cp)
                    xgTj = [cp.tile([P, Cp], bf16, name=f"xgT{j}_{k}",
                                    tag=f"xg{j}_{k}") for k in range(KD)]
                    for ct in range(NCT):
                        for kd in range(KD):
                            pt = pb.tile([P, P], bf16, tag="po", space="PSUM")
                            nc.tensor.transpose(
                                out=pt[:],
                                in_=xgrows[j][ct][:, kd * P:(kd + 1) * P],
                                identity=ident[:])
                            nc.vector.tensor_copy(
                                out=xgTj[kd][:, ct * P:(ct + 1) * P],
                                in_=pt[:])
                    for hf in range(NH):
                        w1h = wp.tile([P, KD, HW], bf16, tag="w1h")
                        nc.sync.dma_start(
                            w1h[:],
                            w1s[j, :, hf * HW:(hf + 1) * HW]
                            .rearrange("(k p) n -> p k n", p=P))
                        w3h = wp.tile([P, KD, HW], bf16, tag="w3h")
                        nc.sync.dma_start(
                            w3h[:],
                            w3s[j, :, hf * HW:(hf + 1) * HW]
                            .rearrange("(k p) n -> p k n", p=P))
                        w2h = wp.tile([P, HTH, D], bf16, tag="w2h")
                        nc.sync.dma_start(
                            w2h[:],
                            w2s[j, hf * HW:(hf + 1) * HW, :]
                            .rearrange("(h p) n -> p h n", p=P))
                        for cb in range(NCB):
                            yts = []
                            for ht in range(HTH):
                                p1 = pa.tile([P, CB], f32, tag="p1",
                                             space="PSUM")
                                p3 = pa.tile([P, CB], f32, tag="p3",
                                             space="PSUM")
                                for k in range(KD):
                                    nc.tensor.matmul(
                                        out=p1[:],
                                        lhsT=w1h[:, k, ht * P:(ht + 1) * P],
                                        rhs=xgTj[k][:, cb * CB:(cb + 1) * CB],
                                        start=(k == 0), stop=(k == KD - 1))
                                for k in range(KD):
                                    nc.tensor.matmul(
                                        out=p3[:],
                                        lhsT=w3h[:, k, ht * P:(ht + 1) * P],
                                        rhs=xgTj[k][:, cb * CB:(cb + 1) * CB],
                                        start=(k == 0), stop=(k == KD - 1))
                                yt = yp.tile([P, CB], bf16, tag="yt")
                                nc.scalar.activation(out=yt[:], in_=p1[:],
                                                     func=Act.Silu)
                                nc.vector.tensor_mul(out=yt[:], in0=yt[:],
                                                     in1=p3[:])
                                yts.append(yt)
                            for ts in range(TS):
                                po = pb.tile([P, D], f32, tag="po",
                                             space="PSUM")
                                for ht in range(HTH):
                                    nc.tensor.matmul(
                                        out=po[:],
                                        lhsT=yts[ht][:, ts * P:(ts + 1) * P],
                                        rhs=w2h[:, ht, :],
                                        start=(ht == 0), stop=(ht == HTH - 1))
                                cti = cb * TS + ts
                                if hf == 0:
                                    nc.vector.tensor_copy(out=og[j][cti][:],
                                                          in_=po[:])
                                else:
                                    nc.vector.tensor_add(out=og[j][cti][:],
                                                         in0=og[j][cti][:],
                                                         in1=po[:])
                                    # final value: scatter rows to their
                                    # token positions in the dense staging
                                    # (pad rows rewrite token 0 with its own
                                    # values -- harmless collision)
                                    nc.gpsimd.indirect_dma_start(
                                        out=sg[j][:, :],
                                        out_offset=bass.IndirectOffsetOnAxis(
                                            ap=fwsl[j][:, cti:cti + 1],
                                            axis=0),
                                        in_=og[j][cti][:], in_offset=None,
                                        bounds_check=N - 1, oob_is_err=False)
                # combine: read both experts' dense stagings chunk by chunk,
                # weight, and reduce-scatter across cores.
                for c in range(CH):
                    for tt in range(CHW // P):
                        ti = c * (CHW // P) + tt
                        g0 = gp.tile([P, D], bf16, tag="g0")
                        nc.sync.dma_start(g0[:], sg[0][ti * P:(ti + 1) * P, :])
                        g1 = gp.tile([P, D], bf16, tag="g1")
                        nc.sync.dma_start(g1[:], sg[1][ti * P:(ti + 1) * P, :])
                        tw0 = TW[:, ti * E: ti * E + 1]
                        tw1 = TW[:, ti * E + 1: ti * E + 2]
                        nc.vector.tensor_scalar(
                            out=acc[ti][:], in0=g0[:], scalar1=tw0,
                            scalar2=None, op0=Alu.mult)
                        nc.vector.scalar_tensor_tensor(
                            out=acc[ti][:], in0=g1[:], scalar=tw1,
                            in1=acc[ti][:], op0=Alu.mult, op1=Alu.add)
                    shc = CHW // NCORES
                    partc = dp.tile([CHW, D], bf16, name=f"part{c}",
                                    tag=f"part{c}", space="DRAM")
                    outbc = dp.tile([shc, D], bf16, name=f"outb{c}",
                                    tag=f"outb{c}", space="DRAM")
                    for tt in range(CHW // P):
                        nc.sync.dma_start(
                            partc[tt * P:(tt + 1) * P, :],
                            acc[c * (CHW // P) + tt][:])
                    nc.gpsimd.collective_compute(
                        "ReduceScatter", mybir.AluOpType.add,
                        replica_groups=[list(range(NCORES))],
                        ins=[partc[:].opt()], outs=[outbc[:].opt()])
                    nc.sync.dma_start(
                        out_sh[c * shc:(c + 1) * shc, :], outbc[:])


def _build():
    global _PROG
    if _PROG is not None:
        return _PROG
    import concourse.mybir as mybir
    import concourse.tile as tile
    from concourse import bacc

    nc = bacc.Bacc("TRN2", target_bir_lowering=False, debug=False,
                   enable_asserts=True, num_devices=NCORES)
    f32 = mybir.dt.float32
    bf16 = mybir.dt.bfloat16
    i32 = mybir.dt.int32
    xt = nc.dram_tensor("xt", [D, N], f32, kind="ExternalInput").ap()
    xrow16 = nc.dram_tensor("xrow16", [N, D], bf16, kind="ExternalInput").ap()
    tokid = nc.dram_tensor("tokid", [P, N // P], i32, kind="ExternalInput").ap()
    wcat = nc.dram_tensor("wcat", [D, G + E], f32, kind="ExternalInput").ap()
    bcat = nc.dram_tensor("bcat", [1, G + E], f32, kind="ExternalInput").ap()
    w1s = nc.dram_tensor("w1s", [EPC, D, H], bf16, kind="ExternalInput").ap()
    w3s = nc.dram_tensor("w3s", [EPC, D, H], bf16, kind="ExternalInput").ap()
    w2s = nc.dram_tensor("w2s", [EPC, H, D], bf16, kind="ExternalInput").ap()
    out_sh = nc.dram_tensor("out_shard", [SHARD, D], bf16,
                            kind="ExternalOutput").ap()
    with tile.TileContext(nc) as tc:
        _emit(tc, xt, xrow16, tokid, wcat, bcat, w1s, w3s, w2s, out_sh)
    nc.compile()
    _PROG = nc
    return nc


def _host_in_maps(x, Wr, br, Wgate, bgate, W1, W3, W2):
    x = np.asarray(x, np.float32)
    Wr = np.asarray(Wr, np.float32)
    br = np.asarray(br, np.float32)
    Wgate = np.asarray(Wgate, np.float32)
    bgate = np.asarray(bgate, np.float32)
    W1 = np.asarray(W1, np.float32)
    W3 = np.asarray(W3, np.float32)
    W2 = np.asarray(W2, np.float32)

    import ml_dtypes
    xt = np.ascontiguousarray(x.reshape(N, D).T)  # (D, N)
    xrow16 = x.reshape(N, D).astype(ml_dtypes.bfloat16)
    tokid = (np.arange(N // P, dtype=np.int32)[None, :] * P
             + np.arange(P, dtype=np.int32)[:, None])
    tokid = np.ascontiguousarray(tokid)
    in_maps = []
    for c in range(NCORES):
        g = c // 2
        e0 = (2 * c) % EPG
        gperm = [g] + [gg for gg in range(G) if gg != g]
        eperm = [e0, e0 + 1] + [ee for ee in range(EPG)
                                if ee not in (e0, e0 + 1)]
        gate_cols = []
        gate_bias = []
        for si, gg in enumerate(gperm):
            ep = eperm if si == 0 else list(range(EPG))
            gate_cols.append(Wgate[gg][:, ep])
            gate_bias.append(bgate[gg][ep])
        wcat = np.ascontiguousarray(
            np.concatenate([Wr[:, gperm]] + gate_cols, axis=1))  # (D, 20)
        bcat = np.ascontiguousarray(
            np.concatenate([br[gperm]] + gate_bias)[None, :])    # (1, 20)
        es = [2 * c, 2 * c + 1]
        in_maps.append({
            "xt": xt,
            "xrow16": xrow16,
            "tokid": tokid,
            "wcat": wcat,
            "bcat": bcat,
            "w1s": np.ascontiguousarray(W1[es]).astype(ml_dtypes.bfloat16),
            "w3s": np.ascontiguousarray(W3[es]).astype(ml_dtypes.bfloat16),
            "w2s": np.ascontiguousarray(W2[es]).astype(ml_dtypes.bfloat16),
        })
    return in_maps


def kernel(x, Wr, br, Wgate, bgate, W1, W3, W2):
    global LAST_EXEC_NS, LAST_TRACE
    from concourse.bass_utils import run_bass_kernel_spmd

    nc = _build()
    in_maps = _host_in_maps(x, Wr, br, Wgate, bgate, W1, W3, W2)
    trace = bool(int(os.environ.get("KERNEL_TRACE", "0")))
    res = run_bass_kernel_spmd(nc, in_maps, list(range(NCORES)), trace=trace)
    LAST_EXEC_NS = res.exec_time_ns
    LAST_TRACE = res.instructions_and_trace
    # out_shard rows [c*64:(c+1)*64] on core r are global tokens
    # c*CHW + r*64 + [0, 64): undo the chunked reduce-scatter interleave.
    shc = CHW // NCORES
    out = np.empty((N, D), np.float32)
    for r in range(NCORES):
        sh = res.results[r]["out_shard"].reshape(CH, shc, D)
        for c in range(CH):
            out[c * CHW + r * shc: c * CHW + (r + 1) * shc] = sh[c]
    return out.reshape(B, T, D).astype(np.float32)
